# revision 41
# baseline (speedup 1.0000x reference)
"""Trainium2 Bass kernel for nn_MultiHeadAttention (B=2, S=2048, D=1024, H=16).

Sharding: 8 cores = 2 (batch) x 4 (head groups of 4 heads / 256 proj dims).
Each core computes q/k/v projections for its 256-dim slice, attention for its
4 heads, and a partial out-projection y_part = attn_out @ Wo[slice].  The host
gather sums the 4 partials per batch (bo is added on one core per group via a
zeros-bias trick so the program stays SPMD-uniform).

Design (v2):
 - x is cast to bf16 and PE-transposed (1 cyc/row); xT stored as fp8 e4m3.
 - q/k/v projections run in fp8 with DoubleRow perf mode (2 contraction rows
   per pass -> half the matmuls).  Weights are scaled x1024 before e4m3
   quantization (biases pre-scaled on host); the bias-add rescales by 2^-10.
 - Scores stay bf16 ([k, q] layout, K=64 row-split per head pair); exp on the
   Activation engine; PV and ones-rowsum matmuls stay bf16, col-packed in
   pairs via tile_position.  ones=1024 folds the V-scale rescale into the
   softmax denominator for free.
 - Phase A interleaves transposes and K/Q projection per 512-seq group so the
   PE stays dense while x DMAs stream; V projection and the out-projection
   are PE filler inside the attention loop.
 - Block end: pv psum staged out on GpSimd; denominator reciprocal (DVE) runs
   double-buffered (sum psum bufs=2) off the critical path.
"""

import sys

sys.path.insert(0, "/opt/trn_rl_repo")

import numpy as np

import concourse.bass as bass
import concourse.mybir as mybir
import concourse.tile as _tile_mod
from concourse.masks import make_identity
from concourse.tile import TileContext
from concourse.vector_clock import ScopedClock


def _drain_and_barrier_split_waits(self, tick_clock, wait_clock):
    """Replacement for TileContext._drain_and_barrier.

    The walrus build in this container only accepts one sync-wait command per
    CTRL instruction; the stock tail drain carries one wait per outstanding
    proc and fails codegen with "Too many sync wait commands".  Attach the
    waits to a nop first, then redistribute the surplus onto extra nops.
    """
    carrier = self.nc.sync.nop()
    wait_clock.add_sem_waits(carrier.ins, ScopedClock({None: tick_clock.global_clock}))
    si = carrier.ins.sync_info
    if si is not None and len(si.on_wait) > 1:
        waits = list(si.on_wait)
        carrier.ins.sync_info = mybir.SyncInfo(
            on_wait=[waits[0]], on_update=list(si.on_update)
        )
        for w in waits[1:]:
            extra = self.nc.sync.nop()
            extra.ins.sync_info = mybir.SyncInfo(on_wait=[w], on_update=[])
    self.nc.sync.drain()

    self.nc.all_engine_barrier()
    assert self.sems is not None
    popped = self.nc._tile_sem_poison_stack.pop()
    assert popped is self._sem_poison
    self.nc.clear_and_free_semaphores(list(self.sems.allocated().values()))
    self.nc.all_engine_barrier()


_tile_mod.TileContext._drain_and_barrier = _drain_and_barrier_split_waits


def _split_excess_waits(nc):
    """This container's walrus accepts only ONE sync-wait command per
    instruction.  Tile emits up to 3.  Hoist all but the last wait of each
    instruction onto fresh same-engine NoOps placed directly before it --
    sound because walrus lowers DMA waits into the issuing sequencer's
    pseudo-instruction, so waits always gate the same sequencer stream."""
    ctr = 0
    for fn in nc.m.functions:
        for blk in fn.blocks:
            rewritten = []
            changed = False
            for ins in blk.instructions:
                si = ins.sync_info
                if si is not None and len(si.on_wait) > 1:
                    waits = list(si.on_wait)
                    for w in waits[:-1]:
                        nop = mybir.InstNoOp(name=f"I-wsplit-{ctr}", ins=[], outs=[])
                        ctr += 1
                        nop.engine = ins.engine
                        nop.sync_info = mybir.SyncInfo(on_wait=[w], on_update=[])
                        nc.register_instruction(nop)
                        rewritten.append(nop)
                    ins.sync_info = mybir.SyncInfo(
                        on_wait=[waits[-1]], on_update=list(si.on_update)
                    )
                    changed = True
                rewritten.append(ins)
            if changed:
                blk.instructions = rewritten
    return nc


F32 = mybir.dt.float32
BF16 = mybir.dt.bfloat16
E4M3 = mybir.dt.float8e4
DR = mybir.MatmulPerfMode.DoubleRow
ADD = mybir.AluOpType.add
MULT = mybir.AluOpType.mult
EXP = mybir.ActivationFunctionType.Exp

P = 128
D_MODEL = 1024
N_HEADS = 16
HEAD_DIM = 64
SCALE = HEAD_DIM**-0.5

# per-core sizes
NL = 256  # local projection dims (4 heads x 64)
HL = 4  # local heads
QBS = 512  # q block size for attention
WS = 1024.0  # weight quantization pre-scale (biases host-scaled to match)


def build_bass(S: int) -> bass.Bass:
    """One SPMD program; every core runs it on its own shard."""
    D = D_MODEL
    DC = D // P  # d chunks (8)
    SC = S // P  # s chunks
    QB = S // QBS  # q blocks
    KC = S // P  # k chunks

    nc = bass.Bass()
    x = nc.declare_dram_parameter("x", [S, D], F32, isOutput=False)
    wq = nc.declare_dram_parameter("wq", [D, NL], F32, isOutput=False)
    wk = nc.declare_dram_parameter("wk", [D, NL], F32, isOutput=False)
    wv = nc.declare_dram_parameter("wv", [D, NL], F32, isOutput=False)
    bq = nc.declare_dram_parameter("bq", [NL], F32, isOutput=False)  # x1024 on host
    bk = nc.declare_dram_parameter("bk", [NL], F32, isOutput=False)  # x1024
    bv = nc.declare_dram_parameter("bv", [NL], F32, isOutput=False)  # x1024
    wo = nc.declare_dram_parameter("wo", [NL, D], F32, isOutput=False)
    bo = nc.declare_dram_parameter("bo", [D], F32, isOutput=False)
    y = nc.declare_dram_parameter("y", [S, D], F32, isOutput=True)

    with TileContext(nc) as tc:
        with (
            tc.tile_pool(name="persist", bufs=1) as pp,
            tc.tile_pool(name="stage", bufs=3) as stage,
            tc.tile_pool(name="expp", bufs=3) as expp,
            tc.tile_pool(name="small", bufs=3) as small,
        ):
            # ---- constants ----
            ident = pp.tile([P, P], F32, name="ident")
            make_identity(nc, ident)
            ident_b = pp.tile([P, P], BF16, name="ident_b")
            nc.vector.tensor_copy(ident_b, ident)
            # ones=1024 folds the x1024 V scale out via the softmax denominator
            ones = pp.tile([P, HEAD_DIM], BF16, name="ones")
            nc.vector.memset(ones, WS)

            # ---- persistent activations ----
            xT8 = pp.tile([P, DC, S], E4M3, name="xT8")  # [d_in_chunk, dc, s]
            QT = pp.tile([P, 2, S], BF16, name="QT")  # [n_in_chunk, nchunk, s]
            KT = pp.tile([P, 2, S], BF16, name="KT")
            V = pp.tile([P, SC, HL, HEAD_DIM], BF16, name="V")  # holds 1024*v
            outT = pp.tile([P, 2, S], BF16, name="outT")  # [n_in_chunk, hp, q]

            # ---- biases (bv arrives x1024 from host; bq/bk raw) ----
            bq_sb = pp.tile([P, 2], F32, name="bq_sb")
            nc.sync.dma_start(bq_sb, bq[:].rearrange("(o p) -> p o", p=P))
            bk_sb = pp.tile([P, 2], F32, name="bk_sb")
            nc.sync.dma_start(bk_sb, bk[:].rearrange("(o p) -> p o", p=P))
            bv_sb = pp.tile([P, NL], F32, name="bv_sb")
            nc.sync.dma_start(bv_sb, bv[:].unsqueeze(0).to_broadcast((P, NL)))
            bo_sb = pp.tile([P, D], F32, name="bo_sb")
            nc.sync.dma_start(bo_sb, bo[:].unsqueeze(0).to_broadcast((P, D)))

            # ---- weights: qkv -> e4m3 (x1024), wo -> bf16 ----
            # wq/wk casts up front (gate the phase-A projections); wv/wo are
            # deferred past the sg loop (only attention-phase filler needs
            # them) so the DVE stays clear for the xT copies.
            wq8 = pp.tile([P, DC, NL], E4M3, name="wq8")
            wk8 = pp.tile([P, DC, NL], E4M3, name="wk8")
            wv8 = pp.tile([P, DC, NL], E4M3, name="wv8")
            wo_bf = pp.tile([P, 2, D], BF16, name="wo_bf")
            _x_stage_tiles = []
            for _i in range(8):
                _xt = stage.tile([P, D], F32, tag="x", bufs=8)
                _eng = (nc.sync, nc.scalar, nc.gpsimd, nc.scalar)[_i % 4]
                _eng.dma_start(_xt, x[_i * P : (_i + 1) * P, :])
                _x_stage_tiles.append(_xt)
            wqueues = (nc.scalar, nc.gpsimd, nc.sync)
            wv_stage = []
            wkq_stage = {}
            for wi, (w_dram, w8) in enumerate(
                ((wk, wk8), (wq, wq8), (wv, wv8))
            ):
                w3 = w_dram[:].rearrange("(c p) n -> p c n", p=P)
                for dc in range(DC):
                    wst = stage.tile([P, NL], F32, tag="wst", bufs=12)
                    wqueues[(wi * DC + dc) % 3].dma_start(wst, w3[:, dc, :])
                    if wi < 2:
                        wkq_stage[(wi, dc)] = wst
                    else:
                        wv_stage.append((wst, dc))
            wo3 = wo[:].rearrange("(c p) n -> p c n", p=P)
            wo_stage = []
            for nch in range(2):
                wst2 = stage.tile([P, D], F32, tag="wst2")
                nc.scalar.dma_start(wst2, wo3[:, nch, :])
                wo_stage.append((wst2, nch))

            def qk_piece(pool, tag_bufs, w8, b_sb, dest, nsub, sb):
                # one [128, 512] slice of QT/KT: 4 fp8 DoubleRow matmuls
                ps = pool.tile([P, 512], F32, tag=tag_bufs[0], bufs=tag_bufs[1], name="ps_qk")
                for g in range(DC // 2):
                    nc.tensor.matmul(
                        ps,
                        lhsT=w8[:, 2 * g : 2 * g + 2, nsub * P : (nsub + 1) * P],
                        rhs=xT8[:, 2 * g : 2 * g + 2, sb * 512 : (sb + 1) * 512],
                        start=(g == 0),
                        stop=(g == DC // 2 - 1),
                        perf_mode=DR,
                    )
                # bias-add + 2^-10 rescale on the Activation engine
                nc.scalar.activation(
                    dest[:, nsub, sb * 512 : (sb + 1) * 512],
                    ps,
                    mybir.ActivationFunctionType.Identity,
                    bias=b_sb[:, nsub : nsub + 1],
                    scale=1.0 / WS,
                )

            # ---- phase A: x load + bf16 cast + PE transpose + full KT/QT ----
            with tc.tile_pool(name="psA", bufs=1, space="PSUM") as psA:
                NSG = SC // 4
                xts = [None] * (NSG * 4)

                def issue_x_dma(sg):
                    for j in range(4):
                        i = sg * 4 + j
                        xt = stage.tile([P, D], F32, tag="x", bufs=8)
                        eng = (nc.sync, nc.scalar, nc.gpsimd, nc.scalar)[j]
                        eng.dma_start(xt, x[i * P : (i + 1) * P, :])
                        xts[i] = xt

                for _i, _xt in enumerate(_x_stage_tiles):
                    xts[_i] = _xt
                for sg in range(NSG):
                    if sg + 2 < NSG:
                        issue_x_dma(sg + 2)
                    xbs = [xts[sg * 4 + j] for j in range(4)]
                    for dc in range(DC):
                        tp = psA.tile([P, 4, P], F32, tag="tp", bufs=3)
                        for j in range(4):
                            nc.tensor.transpose(
                                tp[:, j, :], xbs[j][:, dc * P : (dc + 1) * P], ident
                            )
                        # f32 psum -> e4m3 xT, alternating DVE / ACT
                        if dc % 2 == 0:
                            nc.vector.tensor_copy(
                                xT8[:, dc, sg * 512 : (sg + 1) * 512], tp
                            )
                        else:
                            nc.scalar.copy(
                                xT8[:, dc, sg * 512 : (sg + 1) * 512], tp
                            )
                    if sg == 0:
                        # weight casts only now: keeps the DVE queue free for
                        # sg0's psum copies while the w DMAs stream in behind
                        # the x tiles.
                        for dc in range(DC):
                            nc.vector.tensor_scalar(wk8[:, dc, :], wkq_stage[(0, dc)], WS, None, MULT)
                            nc.vector.tensor_scalar(wq8[:, dc, :], wkq_stage[(1, dc)], WS, None, MULT)
                    # K/Q projection for this 512-seq block right away: keeps
                    # the PE dense and ramped while the next x tiles stream in.
                    for nsub in range(2):
                        qk_piece(psA, ("proj", 4), wk8, bk_sb, KT, nsub, sg)
                    for nsub in range(2):
                        qk_piece(psA, ("proj", 4), wq8, bq_sb, QT, nsub, sg)

                # deferred wv/wo casts (DVE) - overlap with attention start
                for wst, dc in wv_stage:
                    nc.vector.tensor_scalar(wv8[:, dc, :], wst, WS, None, MULT)
                for wst2, nch in wo_stage:
                    nc.vector.tensor_copy(wo_bf[:, nch, :], wst2)

            # ---- phase B: attention (scores transposed [k, q]) ----
            with tc.tile_pool(name="psB", bufs=1, space="PSUM") as psB:

                def v_piece(sc):
                    ps = psB.tile([P, 512], F32, tag="gen", bufs=1, name="ps_v")
                    psv = ps[:, :NL]
                    for g in range(DC // 2):
                        nc.tensor.matmul(
                            psv,
                            lhsT=xT8[:, 2 * g : 2 * g + 2, sc * P : (sc + 1) * P],
                            rhs=wv8[:, 2 * g : 2 * g + 2, :],
                            start=(g == 0),
                            stop=(g == DC // 2 - 1),
                            perf_mode=DR,
                        )
                    # V holds 1024*v (psum and host-side bias both pre-scaled)
                    nc.vector.tensor_tensor(
                        V[:, sc],
                        psv.rearrange("p (h d) -> p h d", h=HL),
                        bv_sb.rearrange("p (h d) -> p h d", h=HL),
                        ADD,
                    )

                def y_piece(qc, mb, tag="gen"):
                    if tag == "s":
                        # drain-phase only: the score-staging banks are free
                        # by then and give the evacuations a third buffer
                        pst = psB.tile([P, 2, QBS], F32, tag="s", bufs=2, name="ps_y2")
                        psy = pst[:, 0, :]
                    else:
                        psy = psB.tile([P, 512], F32, tag=tag, bufs=1, name="ps_y")
                    for nch in range(2):
                        nc.tensor.matmul(
                            psy,
                            lhsT=outT[:, nch, qc * P : (qc + 1) * P],
                            rhs=wo_bf[:, nch, mb * 512 : (mb + 1) * 512],
                            start=(nch == 0),
                            stop=(nch == 1),
                        )
                    yt = small.tile([P, 512], F32, tag="yt")
                    nc.vector.tensor_tensor(yt, psy, bo_sb[:, mb * 512 : (mb + 1) * 512], ADD)
                    (nc.sync if mb == 0 else nc.gpsimd).dma_start(
                        y[qc * P : (qc + 1) * P, mb * 512 : (mb + 1) * 512], yt
                    )

                # Software pipeline across head-pair blocks: block X's score
                # loop is ACT-paced (exp drains one psa group per ~2.2us), so
                # the PE would idle between score groups.  We interleave the
                # PREVIOUS block's pv/rowsum chunks (and V/y filler) into the
                # score loop to fill those waits; each block's pv accumulation
                # is emitted kc-pair by kc-pair as the next block's scores
                # stream.
                class Blk:
                    def __init__(self, qb, hp):
                        self.qb, self.hp = qb, hp
                        self.expA = expp.tile([P, KC, QBS], BF16, tag="exp", bufs=4)
                        self.expB = expp.tile([P, KC, QBS], BF16, tag="exp", bufs=4)
                        self.pv = None
                        self.sm = None

                    def pv_chunk(self, g):
                        # pv/rowsum for kc = 2g, 2g+1 (4 col-packed slots)
                        if g == 0:
                            self.pv = psB.tile([P, QBS], F32, tag="pv", bufs=1, name="pv")
                            self.sm = psB.tile([P, QBS], F32, tag="sum", bufs=2, name="sm")
                        hA, hB = 2 * self.hp, 2 * self.hp + 1
                        for kc in (2 * g, 2 * g + 1):
                            st, sp = (kc == 0), (kc == KC - 1)
                            nc.tensor.matmul(
                                self.pv[0:HEAD_DIM],
                                lhsT=V[:, kc, hA, :],
                                rhs=self.expA[:, kc, :],
                                start=st, stop=sp,
                                skip_group_check=True, tile_position=(0, 0),
                            )
                            nc.tensor.matmul(
                                self.pv[HEAD_DIM:P],
                                lhsT=V[:, kc, hB, :],
                                rhs=self.expB[:, kc, :],
                                start=st, stop=sp,
                                skip_group_check=True, tile_position=(0, 64),
                            )
                            # ones lhsT (M=64, value 1024) replicates each
                            # head's rowsum across its 64 psum partitions
                            nc.tensor.matmul(
                                self.sm[0:HEAD_DIM],
                                lhsT=ones,
                                rhs=self.expA[:, kc, :],
                                start=st, stop=sp,
                                skip_group_check=True, tile_position=(0, 0),
                            )
                            nc.tensor.matmul(
                                self.sm[HEAD_DIM:P],
                                lhsT=ones,
                                rhs=self.expB[:, kc, :],
                                start=st, stop=sp,
                                skip_group_check=True, tile_position=(0, 64),
                            )

                    def normalize(self):
                        # pv staged out of psum immediately; reciprocal is
                        # slow (3.3us) but sum bufs=2 keeps it off the
                        # critical path.
                        pvs = small.tile([P, QBS], F32, tag="pvs")
                        nc.vector.tensor_copy(pvs, self.pv)
                        rbc = small.tile([P, QBS], F32, tag="rbc")
                        nc.vector.reciprocal(rbc, self.sm)
                        nc.vector.tensor_tensor(
                            outT[:, self.hp, self.qb * QBS : (self.qb + 1) * QBS],
                            pvs, rbc, MULT,
                        )

                v_fill = [(lambda sc=sc: v_piece(sc)) for sc in range(SC)][::-1]
                # y(qb) becomes emission-safe only once normalize(qb, hp=1)
                # has been EMITTED - that happens at the end of block
                # (qb+1, hp=0), so promote y_next into y_fill when starting
                # (qb+1, hp=1).
                y_next, y_fill = [], []
                prev = None
                for qb in range(QB):
                    for hp in range(2):
                        if hp == 1 and y_next:
                            y_fill = y_next + y_fill
                            y_next = []
                        blk = Blk(qb, hp)
                        qA = QT[0:HEAD_DIM, hp, qb * QBS : (qb + 1) * QBS]
                        qB = QT[HEAD_DIM:P, hp, qb * QBS : (qb + 1) * QBS]
                        for g in range(KC // 2):
                            psa = psB.tile([P, 2, QBS], F32, tag="s", bufs=2)
                            psb = psB.tile([P, 2, QBS], F32, tag="s", bufs=2)
                            for j in range(2):
                                kc = 2 * g + j
                                # row-packed pair: head A on PE rows 0-63,
                                # head B on rows 64-127 (auto tile_position)
                                nc.tensor.matmul(
                                    psa[:, j],
                                    lhsT=KT[0:HEAD_DIM, hp, kc * P : (kc + 1) * P],
                                    rhs=qA,
                                    start=True, stop=True,
                                )
                                nc.tensor.matmul(
                                    psb[:, j],
                                    lhsT=KT[HEAD_DIM:P, hp, kc * P : (kc + 1) * P],
                                    rhs=qB,
                                    start=True, stop=True,
                                )
                            nc.scalar.activation(blk.expA[:, 2 * g : 2 * g + 2], psa, EXP, scale=SCALE)
                            nc.scalar.activation(blk.expB[:, 2 * g : 2 * g + 2], psb, EXP, scale=SCALE)
                            # fill the ACT-paced wait with useful PE work
                            if prev is not None:
                                prev.pv_chunk(g)
                            else:
                                for _ in range(2):  # block 0: V pieces
                                    if v_fill:
                                        v_fill.pop()()
                            if g >= 2:
                                for _ in range(2):
                                    if y_fill:
                                        qc, mb = y_fill.pop()
                                        y_piece(qc, mb)
                        if prev is not None:
                            prev.normalize()
                        prev = blk
                    y_next = [
                        (qc, mb)
                        for qc in range(qb * (QBS // P), (qb + 1) * (QBS // P))
                        for mb in range(2)
                    ][::-1]

                # drain: last block's pv, then a fine-grained normalize that
                # releases outT in 128-column chunks so the final y pieces
                # start as early as possible (they are the serial tail).
                for g in range(KC // 2):
                    prev.pv_chunk(g)
                rest = y_next + y_fill  # pop order: qc ascending, qb3 last
                rest.reverse()
                early = [t for t in rest if t[0] < 12]
                late = [t for t in rest if t[0] >= 12]
                k = 0
                for qc, mb in early:
                    y_piece(qc, mb, tag=("gen" if k % 2 == 0 else "s"))
                    k += 1
                qb3 = prev.qb * QBS
                for c in range(4):
                    cs = slice(c * P, (c + 1) * P)
                    rbc = small.tile([P, P], F32, tag="rbc2", bufs=2)
                    nc.vector.reciprocal(rbc, prev.sm[:, cs])
                    nc.vector.tensor_tensor(
                        outT[:, prev.hp, qb3 + c * P : qb3 + (c + 1) * P],
                        prev.pv[:, cs], rbc, MULT,
                    )
                    for qc, mb in late:
                        if qc == 12 + c:
                            y_piece(qc, mb, tag=("gen" if k % 2 == 0 else "s"))
                            k += 1

    _split_excess_waits(nc)
    return nc


def shard_inputs(x, Wq, bq, Wk, bk, Wv, bv, Wo, bo):
    """Split full inputs into 8 per-core maps: core c -> (batch c//4, heads slice c%4).

    bv is pre-scaled x1024 to match the on-device weight quantization scale
    (the V bias-add keeps the psum scale; the ones=1024 rowsum cancels it).
    """
    in_maps = []
    zeros_bo = np.zeros_like(bo)
    for c in range(8):
        b, g = c // 4, c % 4
        n0 = g * NL
        in_maps.append(
            {
                "x": np.ascontiguousarray(x[b]),
                "wq": np.ascontiguousarray(Wq[:, n0 : n0 + NL]),
                "wk": np.ascontiguousarray(Wk[:, n0 : n0 + NL]),
                "wv": np.ascontiguousarray(Wv[:, n0 : n0 + NL]),
                "bq": np.ascontiguousarray(bq[n0 : n0 + NL]),
                "bk": np.ascontiguousarray(bk[n0 : n0 + NL]),
                "bv": np.ascontiguousarray(bv[n0 : n0 + NL] * np.float32(WS)),
                "wo": np.ascontiguousarray(Wo[n0 : n0 + NL, :]),
                "bo": bo if g == 0 else zeros_bo,
            }
        )
    return in_maps


_NC_CACHE = {}


def kernel(x, Wq, bq, Wk, bk, Wv, bv, Wo, bo, trace=False, tmpdir=None):
    from concourse.bass_utils import run_bass_kernel_spmd

    x = np.asarray(x, dtype=np.float32)
    args = [np.asarray(a, dtype=np.float32) for a in (Wq, bq, Wk, bk, Wv, bv, Wo, bo)]
    B, S, D = x.shape

    if S not in _NC_CACHE:
        _NC_CACHE[S] = build_bass(S)
    nc = _NC_CACHE[S]

    in_maps = shard_inputs(x, *args)
    res = run_bass_kernel_spmd(
        nc, in_maps, core_ids=list(range(8)), trace=trace, tmpdir=tmpdir
    )
    parts = [np.asarray(res.results[c]["y"]) for c in range(8)]
    out = np.empty((B, S, D), dtype=np.float32)
    for b in range(B):
        out[b] = parts[4 * b] + parts[4 * b + 1] + parts[4 * b + 2] + parts[4 * b + 3]
    if trace:
        kernel.last_result = res
    return out
